# revision 4
# baseline (speedup 1.0000x reference)
"""Trainium2 Bass kernel for nn_BuildVolume2dChaos (bilinear-warp cost volume).

kernel(refimg_fea, targetimg_fea, disps) -> volume [B=2, D=32, H=128, W=256]

Self-contained: builds an SPMD Bass program (one per-core variant), shards
inputs over 8 NeuronCores as (b, h-slice) = (core//4, 32*(core%4)), runs via
concourse.bass_utils.run_bass_kernel_spmd, reassembles the full output.

Algorithm per core (b fixed, 32 h-rows):
  vertical lerp of the target features (grid_sample align_corners=False row
  weights) -> Tv; horizontal bilinear warp expressed as a banded matmul:
  warped[c, (d,w)] = sum_{w'} Tv[c,w'] * relu(1 - |ix(d,w) - w'|) with
  ix = (w - disp)*W/(W-1) - 0.5 and zero-padded Tv. Five 62-wide w-tiles give
  a 128-row w'-window each -> K=128 matmuls on the PE. Tent weights are built
  by broadcasting ix over partitions with a selection matmul and a 2-op tent
  (|.| then clamped affine) on ScalarE or VectorE. |ref - warped| reduces over
  channels with a block-diagonal ones matmul accumulated in PSUM.
"""
import sys

sys.path.insert(0, '/opt/trn_rl_repo')

import numpy as np
import bass_rust
import concourse.bass as bass
import concourse.mybir as mybir
from concourse.tile import TileContext
from concourse.vector_clock import ScopedClock

f32 = mybir.dt.float32
Alu = mybir.AluOpType
ActF = mybir.ActivationFunctionType

B, C, H, W, D = 2, 32, 128, 256, 32
HS = 32
NCORES = 8
BASES = [0, 62, 124, 186, 248]
SIZES = [62, 62, 62, 62, 8]
NT = len(BASES)
TOFF = [62 * k - 65 for k in range(NT)]
XSCALE = W / (W - 1)

_MAX_WAITS = 1


def _split_excess_waits(nc, max_waits=_MAX_WAITS):
    """Walrus (this neuronx-cc XLA path) rejects instructions carrying more
    than ~1 sem-wait ('Too many sync wait commands'). Hoist excess waits onto
    same-engine Drain instructions inserted immediately before."""
    n_fixed = 0
    for f in nc.m.functions:
        for bb in f.blocks:
            insts = bb.instructions
            i = 0
            while i < len(insts):
                ins = insts[i]
                si = ins.sync_info
                if si is not None and si.on_wait and len(si.on_wait) > max_waits:
                    waits = list(si.on_wait)
                    ins.sync_info = bass_rust.SyncInfo(
                        on_wait=waits[:max_waits], on_update=list(si.on_update))
                    pre = []
                    for jj in range(max_waits, len(waits), max_waits):
                        d = mybir.InstDrain(
                            name=f"{ins.name}-ws{jj}", ins=[], outs=[])
                        d.engine = ins.engine
                        d.sync_info = bass_rust.SyncInfo(
                            on_wait=waits[jj:jj + max_waits], on_update=[])
                        pre.append(d)
                    for d in reversed(pre):
                        insts.insert(i, d)
                        nc.register_instruction(d, overwrite=True)
                    i += len(pre)
                    n_fixed += 1
                i += 1
    return n_fixed


class _PatchedTileContext(TileContext):
    """Walrus CoreV3 rejects instructions with >1 sem-wait ('Too many sync
    wait commands'); split the kernel-tail drain's waits across drains."""

    def __exit__(self, exc_type, exc_val, exc_tb):
        ret = super().__exit__(exc_type, exc_val, exc_tb)
        if exc_type is None:
            _split_excess_waits(self.nc)
        return ret

    def _drain_and_barrier(self, tick_clock, wait_clock):
        nc = self.nc
        drain_inst = nc.sync.drain()
        wait_clock.add_sem_waits(
            drain_inst.ins, ScopedClock({None: tick_clock.global_clock})
        )
        si = drain_inst.ins.sync_info
        if si is not None and si.on_wait and len(si.on_wait) > _MAX_WAITS:
            waits = list(si.on_wait)
            drain_inst.ins.sync_info = bass_rust.SyncInfo(
                on_wait=waits[:_MAX_WAITS], on_update=list(si.on_update)
            )
            for i in range(_MAX_WAITS, len(waits), _MAX_WAITS):
                extra = nc.sync.drain()
                extra.ins.sync_info = bass_rust.SyncInfo(
                    on_wait=waits[i: i + _MAX_WAITS], on_update=[]
                )
        nc.all_engine_barrier()
        assert self.sems is not None
        popped = nc._tile_sem_poison_stack.pop()
        assert popped is self._sem_poison
        nc.clear_and_free_semaphores(list(self.sems.allocated().values()))
        nc.all_engine_barrier()


def build_nc(act8=5, reps=1):
    nc = bass.Bass("TRN2", debug=False, enable_asserts=False)

    dispst = nc.dram_tensor("dispst", [HS, D, W], f32, kind="ExternalInput")
    wrow = nc.dram_tensor("wrow", [HS, W], f32, kind="ExternalInput")
    tga = [nc.dram_tensor(f"tga{k}", [128, HS, C], f32, kind="ExternalInput")
           for k in range(NT)]
    tgb = [nc.dram_tensor(f"tgb{k}", [128, HS, C], f32, kind="ExternalInput")
           for k in range(NT)]
    wyb = nc.dram_tensor("wyb", [128, HS], f32, kind="ExternalInput")
    refrep = nc.dram_tensor("refrep", [128, HS, W], f32, kind="ExternalInput")
    selpad = nc.dram_tensor("selpad", [HS, HS * 128], f32, kind="ExternalInput")
    wpb = nc.dram_tensor("wpb", [128, NT], f32, kind="ExternalInput")
    bdp = nc.dram_tensor("bdp", [128, 256], f32, kind="ExternalInput")
    vol = nc.dram_tensor("vol", [D, HS, W], f32, kind="ExternalOutput")
    vol_v = vol.ap().rearrange("(dq dp) h w -> h dq dp w", dq=4, dp=8)

    with _PatchedTileContext(nc) as tc:
        with (
            tc.tile_pool(name="const", bufs=1) as cpool,
            tc.tile_pool(name="tv", bufs=1) as tvpool,
            tc.tile_pool(name="work", bufs=3) as wpool,
            tc.tile_pool(name="outs", bufs=2) as opool,
            tc.tile_pool(name="pa", bufs=2, space="PSUM") as pa_pool,
            tc.tile_pool(name="pw", bufs=2, space="PSUM") as pw_pool,
            tc.tile_pool(name="po", bufs=2, space="PSUM") as po_pool,
        ):
            s_wrow = cpool.tile([HS, W], f32, tag="wrow")
            nc.sync.dma_start(s_wrow[:, :], wrow[:, :])
            s_wyb = cpool.tile([128, HS], f32, tag="wyb")
            nc.sync.dma_start(s_wyb[:, :], wyb[:, :])
            s_ref = cpool.tile([128, HS, W + 64], f32, tag="ref")
            nc.vector.memset(s_ref[:, :, :], 0.0)
            nc.sync.dma_start(s_ref[:, :, :W], refrep[:, :, :])
            s_sel = cpool.tile([HS, HS * 128], f32, tag="sel")
            nc.sync.dma_start(s_sel[:, :], selpad[:, :])
            s_wpb = cpool.tile([128, NT], f32, tag="wpb")
            nc.sync.dma_start(s_wpb[:, :], wpb[:, :])
            s_bdp = cpool.tile([128, 256], f32, tag="bdp")
            nc.sync.dma_start(s_bdp[:, :], bdp[:, :])
            s_ix = cpool.tile([HS, D, W + 64], f32, tag="ix")
            nc.vector.memset(s_ix[:, :, :], 1.0e6)
            s_tv, s_ntv = [], []
            for k in range(NT):
                s_tv.append(tvpool.tile([128, HS, C], f32, tag=f"tv{k}",
                                        name=f"tv{k}"))
                s_ntv.append(tvpool.tile([128, HS, C], f32, tag=f"ntv{k}",
                                         name=f"ntv{k}"))

            with tc.tile_pool(name="ixp", bufs=1) as xpool:
                DH = D // 2
                wrow_b = s_wrow[:, :].unsqueeze(1).broadcast_to([HS, DH, W])
                for half in range(2):
                    s_disp = xpool.tile([HS, DH * W], f32, tag="disp")
                    nc.sync.dma_start(
                        s_disp[:, :],
                        dispst[:, half * DH:(half + 1) * DH, :].rearrange(
                            "h d w -> h (d w)"))
                    nc.vector.scalar_tensor_tensor(
                        s_ix[:, half * DH:(half + 1) * DH, :W],
                        s_disp[:, :].rearrange("h (d w) -> h d w", d=DH),
                        -XSCALE, wrow_b, Alu.mult, Alu.add)

            with tc.tile_pool(name="lerp", bufs=1) as lpool:
                wyb_b = s_wyb[:, :].unsqueeze(2).broadcast_to([128, HS, C])
                for k in range(NT):
                    ta = lpool.tile([128, HS, C], f32, tag="ta")
                    tb = lpool.tile([128, HS, C], f32, tag="tb")
                    nc.sync.dma_start(ta[:, :, :], tga[k][:, :, :])
                    nc.sync.dma_start(tb[:, :, :], tgb[k][:, :, :])
                    u = lpool.tile([128, HS, C], f32, tag="u")
                    nc.vector.tensor_tensor(u[:, :, :], tb[:, :, :],
                                            ta[:, :, :], Alu.subtract)
                    v = lpool.tile([128, HS, C], f32, tag="v")
                    nc.vector.tensor_tensor(v[:, :, :], u[:, :, :], wyb_b,
                                            Alu.mult)
                    nc.vector.tensor_tensor(s_tv[k][:, :, :], ta[:, :, :],
                                            v[:, :, :], Alu.add)
                    nc.vector.scalar_tensor_tensor(
                        s_ntv[k][:, :, :], v[:, :, :], -1.0, ta[:, :, :],
                        Alu.mult, Alu.subtract)

            NK = 512
            for rep in range(reps):
                for k in range(NT):
                    T = SIZES[k]
                    base = BASES[k]
                    outp = po_pool.tile([128, NK], f32, tag="outp")
                    for g in range(HS):
                        use_act = (g % 8) < act8
                        warped = pw_pool.tile([128, NK], f32, tag="warped")
                        for half in range(2):
                            abig = pa_pool.tile([128, 2 * NK], f32, tag="abig")
                            sel_g = s_sel[:, (HS - 1 - g) * 128:(HS - g) * 128]
                            for j in range(2):
                                dq = 2 * half + j
                                rhs = s_ix[:, dq * 8:(dq + 1) * 8,
                                           base:base + 64]
                                nc.tensor.matmul(abig[:, j * NK:(j + 1) * NK],
                                                 sel_g, rhs,
                                                 start=True, stop=True)
                            tent = wpool.tile([128, 2 * NK], f32, tag="tent")
                            yv = wpool.tile([128, 2 * NK], f32, tag="yv")
                            if use_act:
                                nc.scalar.activation(yv[:, :], abig[:, :],
                                                     ActF.Abs,
                                                     bias=s_wpb[:, k:k + 1],
                                                     scale=1.0)
                                nc.scalar.activation(tent[:, :], yv[:, :],
                                                     ActF.Relu,
                                                     bias=1.0, scale=-1.0)
                                lhs = s_tv[k]
                            else:
                                yv0 = wpool.tile([128, 2 * NK], f32, tag="yv0")
                                nc.vector.tensor_scalar(yv0[:, :], abig[:, :],
                                                        s_wpb[:, k:k + 1], None,
                                                        Alu.add)
                                nc.vector.scalar_tensor_tensor(
                                    yv[:, :], yv0[:, :], -1.0, yv0[:, :],
                                    Alu.mult, Alu.max)
                                nc.vector.tensor_scalar(tent[:, :], yv[:, :],
                                                        1.0, -1.0,
                                                        Alu.min, Alu.add)
                                lhs = s_ntv[k]
                            for j in range(2):
                                dq = 2 * half + j
                                nc.tensor.matmul(
                                    warped[32 * dq:32 * (dq + 1), :],
                                    lhs[:, g, :],
                                    tent[:, j * NK:(j + 1) * NK],
                                    start=True, stop=True,
                                    tile_position=(0, 32 * dq))
                        ref_b = s_ref[:, g:g + 1, base:base + 64].broadcast_to(
                            [128, 8, 64])
                        df = wpool.tile([128, NK], f32, tag="df")
                        nc.vector.scalar_tensor_tensor(
                            df[:, :].rearrange("p (a b) -> p a b", a=8),
                            warped[:, :].rearrange("p (a b) -> p a b", a=8),
                            -1.0, ref_b, Alu.mult, Alu.add)
                        adf = wpool.tile([128, NK], f32, tag="adf")
                        nc.vector.scalar_tensor_tensor(
                            adf[:, :], df[:, :], -1.0, df[:, :],
                            Alu.mult, Alu.max)
                        nc.tensor.matmul(outp[:, :],
                                         s_bdp[:, 128 - 4 * g:256 - 4 * g],
                                         adf[:, :], start=(g == 0),
                                         stop=(g == HS - 1))
                    ov = opool.tile([128, NK], f32, tag="ov")
                    nc.vector.tensor_copy(ov[:, :], outp[:, :])
                    nc.sync.dma_start(
                        vol_v[:, :, :, base:base + T],
                        ov[:, :].rearrange("p (dp w) -> p dp w", dp=8)[:, :, :T])
    return nc


def _vertical_rows():
    h = np.arange(H)
    iy = h * (H / (H - 1)) - 0.5
    y0 = np.floor(iy).astype(int)
    wy1 = (iy - y0).astype(np.float32)
    return y0, wy1


def prep_core_inputs(refimg_fea, targetimg_fea, disps, core):
    b = core // 4
    h0 = HS * (core % 4)
    y0, wy1 = _vertical_rows()
    out = {}
    out["dispst"] = np.ascontiguousarray(
        disps[b, :, h0:h0 + HS, :].transpose(1, 0, 2)).astype(np.float32)
    w = np.arange(W, dtype=np.float32)
    out["wrow"] = np.broadcast_to(w * XSCALE - 0.5, (HS, W)).copy()
    tgt = targetimg_fea[b]
    for k in range(NT):
        wp = TOFF[k] + np.arange(128)
        wvalid = (wp >= 0) & (wp < W)
        ga = np.zeros((128, HS, C), np.float32)
        gb = np.zeros((128, HS, C), np.float32)
        tgt_t = np.ascontiguousarray(tgt.transpose(2, 1, 0))  # [W, H, C]
        gh = h0 + np.arange(HS)
        ra, rb = y0[gh], y0[gh] + 1
        rava = (ra >= 0) & (ra < H)
        rbva = (rb >= 0) & (rb < H)
        ga[np.ix_(wvalid, rava)] = tgt_t[wp[wvalid]][:, ra[rava], :]
        gb[np.ix_(wvalid, rbva)] = tgt_t[wp[wvalid]][:, rb[rbva], :]
        out[f"tga{k}"] = ga
        out[f"tgb{k}"] = gb
    out["wyb"] = np.broadcast_to(wy1[h0:h0 + HS], (128, HS)).copy()
    out["refrep"] = np.tile(refimg_fea[b, :, h0:h0 + HS, :],
                            (4, 1, 1)).astype(np.float32)
    sel = np.zeros((HS, HS * 128), np.float32)
    for k in range(HS):
        sel[k, (HS - 1 - k) * 128:(HS - k) * 128] = 1.0
    out["selpad"] = sel
    p = np.arange(128, dtype=np.float32)
    out["wpb"] = np.stack([-(TOFF[k] + p) for k in range(NT)],
                          axis=1).astype(np.float32)
    bdpm = np.zeros((128, 256), np.float32)
    bdpm[np.arange(128), 128 + np.arange(128) // 32] = 1.0
    out["bdp"] = bdpm
    return out


_NC_CACHE = {}


def _get_nc(act8=5, reps=1):
    key = (act8, reps)
    if key not in _NC_CACHE:
        _NC_CACHE[key] = build_nc(act8=act8, reps=reps)
    return _NC_CACHE[key]


def run(refimg_fea, targetimg_fea, disps, act8=5, reps=1):
    from concourse.bass_utils import run_bass_kernel_spmd
    nc = _get_nc(act8=act8, reps=reps)
    in_maps = [prep_core_inputs(refimg_fea, targetimg_fea, disps, core)
               for core in range(NCORES)]
    res = run_bass_kernel_spmd(nc, in_maps, core_ids=list(range(NCORES)))
    full = np.empty((B, D, H, W), np.float32)
    for core in range(NCORES):
        b = core // 4
        h0 = HS * (core % 4)
        full[b, :, h0:h0 + HS, :] = res.results[core]["vol"]
    return full


def kernel(refimg_fea, targetimg_fea, disps):
    refimg_fea = np.asarray(refimg_fea, dtype=np.float32)
    targetimg_fea = np.asarray(targetimg_fea, dtype=np.float32)
    disps = np.asarray(disps, dtype=np.float32)
    return run(refimg_fea, targetimg_fea, disps)


# revision 10
# speedup vs baseline: 1.5892x; 1.5892x over previous
"""Trainium2 Bass kernel for nn_BuildVolume2dChaos (bilinear-warp cost volume).

kernel(refimg_fea, targetimg_fea, disps) -> volume [B=2, D=32, H=128, W=256]

Self-contained: builds an SPMD Bass program (one per-core variant), shards
inputs over 8 NeuronCores as (b, h-slice) = (core//4, 32*(core%4)), runs via
concourse.bass_utils.run_bass_kernel_spmd, reassembles the full output.

Algorithm per core (b fixed, 32 h-rows):
  vertical lerp of the target features (grid_sample align_corners=False row
  weights) -> Tv; horizontal bilinear warp expressed as a banded matmul:
  warped[c, (d,w)] = sum_{w'} Tv[c,w'] * relu(1 - |ix(d,w) - w'|) with
  ix = (w - disp)*W/(W-1) - 0.5 and zero-padded Tv. Five 62-wide w-tiles give
  a 128-row w'-window each -> K=128 matmuls on the PE. Tent weights are built
  by broadcasting ix over partitions with a selection matmul and a 2-op tent
  (|.| then clamped affine) on ScalarE or VectorE. |ref - warped| reduces over
  channels with a block-diagonal ones matmul accumulated in PSUM.
"""
import sys

sys.path.insert(0, '/opt/trn_rl_repo')

import numpy as np
import bass_rust
import concourse.bass as bass
import concourse.mybir as mybir
from concourse.tile import TileContext
from concourse.vector_clock import ScopedClock

f32 = mybir.dt.float32
Alu = mybir.AluOpType
ActF = mybir.ActivationFunctionType

B, C, H, W, D = 2, 32, 128, 256, 32
HS = 32
NCORES = 8
BASES = [0, 62, 124, 186, 248]
SIZES = [62, 62, 62, 62, 8]
NT = len(BASES)
TOFF = [62 * k - 65 for k in range(NT)]
XSCALE = W / (W - 1)

_MAX_WAITS = 1


def _split_excess_waits(nc, max_waits=_MAX_WAITS):
    """Walrus (this neuronx-cc XLA path) rejects instructions carrying more
    than ~1 sem-wait ('Too many sync wait commands'). Hoist excess waits onto
    same-engine Drain instructions inserted immediately before."""
    n_fixed = 0
    for f in nc.m.functions:
        for bb in f.blocks:
            insts = bb.instructions
            i = 0
            while i < len(insts):
                ins = insts[i]
                si = ins.sync_info
                if si is not None and si.on_wait and len(si.on_wait) > max_waits:
                    waits = list(si.on_wait)
                    ins.sync_info = bass_rust.SyncInfo(
                        on_wait=waits[:max_waits], on_update=list(si.on_update))
                    pre = []
                    for jj in range(max_waits, len(waits), max_waits):
                        d = mybir.InstDrain(
                            name=f"{ins.name}-ws{jj}", ins=[], outs=[])
                        d.engine = ins.engine
                        d.sync_info = bass_rust.SyncInfo(
                            on_wait=waits[jj:jj + max_waits], on_update=[])
                        pre.append(d)
                    for d in reversed(pre):
                        insts.insert(i, d)
                        nc.register_instruction(d, overwrite=True)
                    i += len(pre)
                    n_fixed += 1
                i += 1
    return n_fixed


class _PatchedTileContext(TileContext):
    """Walrus CoreV3 rejects instructions with >1 sem-wait ('Too many sync
    wait commands'); split the kernel-tail drain's waits across drains."""

    def __exit__(self, exc_type, exc_val, exc_tb):
        ret = super().__exit__(exc_type, exc_val, exc_tb)
        if exc_type is None:
            _split_excess_waits(self.nc)
        return ret

    def _drain_and_barrier(self, tick_clock, wait_clock):
        nc = self.nc
        drain_inst = nc.sync.drain()
        wait_clock.add_sem_waits(
            drain_inst.ins, ScopedClock({None: tick_clock.global_clock})
        )
        si = drain_inst.ins.sync_info
        if si is not None and si.on_wait and len(si.on_wait) > _MAX_WAITS:
            waits = list(si.on_wait)
            drain_inst.ins.sync_info = bass_rust.SyncInfo(
                on_wait=waits[:_MAX_WAITS], on_update=list(si.on_update)
            )
            for i in range(_MAX_WAITS, len(waits), _MAX_WAITS):
                extra = nc.sync.drain()
                extra.ins.sync_info = bass_rust.SyncInfo(
                    on_wait=waits[i: i + _MAX_WAITS], on_update=[]
                )
        nc.all_engine_barrier()
        assert self.sems is not None
        popped = nc._tile_sem_poison_stack.pop()
        assert popped is self._sem_poison
        nc.clear_and_free_semaphores(list(self.sems.allocated().values()))
        nc.all_engine_barrier()


def build_nc(act8=8, reps=1):
    nc = bass.Bass("TRN2", debug=False, enable_asserts=False)

    dispst = nc.dram_tensor("dispst", [HS, D, W], f32, kind="ExternalInput")
    wrow = nc.dram_tensor("wrow", [HS, W], f32, kind="ExternalInput")
    tga = [nc.dram_tensor(f"tga{k}", [128, HS, C], f32, kind="ExternalInput")
           for k in range(NT)]
    tgb = [nc.dram_tensor(f"tgb{k}", [128, HS, C], f32, kind="ExternalInput")
           for k in range(NT)]
    wyb = nc.dram_tensor("wyb", [128, HS], f32, kind="ExternalInput")
    refrep = nc.dram_tensor("refrep", [128, HS, W], f32, kind="ExternalInput")
    selpad = nc.dram_tensor("selpad", [HS, HS * 128], f32, kind="ExternalInput")
    wpb = nc.dram_tensor("wpb", [128, NT], f32, kind="ExternalInput")
    bdp = nc.dram_tensor("bdp", [128, 256], f32, kind="ExternalInput")
    vol = nc.dram_tensor("vol", [D, HS, W], f32, kind="ExternalOutput")
    vol_v = vol.ap().rearrange("(dq dp) h w -> h dq dp w", dq=4, dp=8)

    with _PatchedTileContext(nc) as tc:
        with (
            tc.tile_pool(name="const", bufs=1) as cpool,
            tc.tile_pool(name="tv", bufs=1) as tvpool,
            tc.tile_pool(name="work", bufs=3) as wpool,
            tc.tile_pool(name="outs", bufs=2) as opool,
            tc.tile_pool(name="pa", bufs=1, space="PSUM") as pa_pool,
            tc.tile_pool(name="pw", bufs=2, space="PSUM") as pw_pool,
            tc.tile_pool(name="po", bufs=2, space="PSUM") as po_pool,
        ):
            s_wrow = cpool.tile([HS, W], f32, tag="wrow")
            nc.sync.dma_start(s_wrow[:, :], wrow[:, :])
            s_wyb = cpool.tile([128, HS], f32, tag="wyb")
            nc.sync.dma_start(s_wyb[:, :], wyb[:, :])
            s_ref = cpool.tile([128, HS, W + 64], f32, tag="ref")
            nc.vector.memset(s_ref[:, :, :], 0.0)
            nc.sync.dma_start(s_ref[:, :, :W], refrep[:, :, :])
            s_sel = cpool.tile([HS, HS * 128], f32, tag="sel")
            nc.sync.dma_start(s_sel[:, :], selpad[:, :])
            s_wpb = cpool.tile([128, NT], f32, tag="wpb")
            nc.sync.dma_start(s_wpb[:, :], wpb[:, :])
            s_bdp = cpool.tile([128, 256], f32, tag="bdp")
            nc.sync.dma_start(s_bdp[:, :], bdp[:, :])
            s_ix = cpool.tile([HS, D, W + 64], f32, tag="ix")
            nc.vector.memset(s_ix[:, :, :], 1.0e6)
            s_tv, s_ntv = [], []
            for k in range(NT):
                s_tv.append(tvpool.tile([128, HS, C], f32, tag=f"tv{k}",
                                        name=f"tv{k}"))
                if act8 < 8:
                    s_ntv.append(tvpool.tile([128, HS, C], f32, tag=f"ntv{k}",
                                             name=f"ntv{k}"))

            with tc.tile_pool(name="ixp", bufs=1) as xpool:
                DH = D // 2
                wrow_b = s_wrow[:, :].unsqueeze(1).broadcast_to([HS, DH, W])
                for half in range(2):
                    s_disp = xpool.tile([HS, DH * W], f32, tag="disp")
                    nc.sync.dma_start(
                        s_disp[:, :],
                        dispst[:, half * DH:(half + 1) * DH, :].rearrange(
                            "h d w -> h (d w)"))
                    nc.vector.scalar_tensor_tensor(
                        s_ix[:, half * DH:(half + 1) * DH, :W],
                        s_disp[:, :].rearrange("h (d w) -> h d w", d=DH),
                        -XSCALE, wrow_b, Alu.mult, Alu.add)

            with tc.tile_pool(name="lerp", bufs=1) as lpool:
                wyb_b = s_wyb[:, :].unsqueeze(2).broadcast_to([128, HS, C])
                for k in range(NT):
                    ta = lpool.tile([128, HS, C], f32, tag="ta")
                    tb = lpool.tile([128, HS, C], f32, tag="tb")
                    nc.sync.dma_start(ta[:, :, :], tga[k][:, :, :])
                    nc.sync.dma_start(tb[:, :, :], tgb[k][:, :, :])
                    u = lpool.tile([128, HS, C], f32, tag="u")
                    nc.vector.tensor_tensor(u[:, :, :], tb[:, :, :],
                                            ta[:, :, :], Alu.subtract)
                    v = lpool.tile([128, HS, C], f32, tag="v")
                    nc.vector.tensor_tensor(v[:, :, :], u[:, :, :], wyb_b,
                                            Alu.mult)
                    nc.vector.tensor_tensor(s_tv[k][:, :, :], ta[:, :, :],
                                            v[:, :, :], Alu.add)
                    if act8 < 8:
                        nc.vector.scalar_tensor_tensor(
                            s_ntv[k][:, :, :], v[:, :, :], -1.0, ta[:, :, :],
                            Alu.mult, Alu.subtract)

            NK = 512
            for rep in range(reps):
                for k in range(NT):
                    T = SIZES[k]
                    base = BASES[k]
                    outp = po_pool.tile([128, NK], f32, tag="outp")
                    for g in range(HS):
                        use_act = (g % 8) < act8
                        warped = pw_pool.tile([128, NK], f32, tag="warped")
                        abig = pa_pool.tile([128, 4 * NK], f32, tag="abig")
                        sel_g = s_sel[:, (HS - 1 - g) * 128:(HS - g) * 128]
                        for dq in range(4):
                            rhs = s_ix[:, dq * 8:(dq + 1) * 8, base:base + 64]
                            nc.tensor.matmul(abig[:, dq * NK:(dq + 1) * NK],
                                             sel_g, rhs, start=True, stop=True)
                        tent = wpool.tile([128, 4 * NK], f32, tag="tent")
                        yv = wpool.tile([128, 4 * NK], f32, tag="yv")
                        if use_act:
                            nc.scalar.activation(yv[:, :], abig[:, :],
                                                 ActF.Abs,
                                                 bias=s_wpb[:, k:k + 1],
                                                 scale=1.0)
                            nc.scalar.activation(tent[:, :], yv[:, :],
                                                 ActF.Relu,
                                                 bias=1.0, scale=-1.0)
                            lhs = s_tv[k]
                        else:
                            yv0 = wpool.tile([128, 4 * NK], f32, tag="yv0")
                            nc.vector.tensor_scalar(yv0[:, :], abig[:, :],
                                                    s_wpb[:, k:k + 1], None,
                                                    Alu.add)
                            nc.vector.scalar_tensor_tensor(
                                yv[:, :], yv0[:, :], -1.0, yv0[:, :],
                                Alu.mult, Alu.max)
                            nc.vector.tensor_scalar(tent[:, :], yv[:, :],
                                                    1.0, -1.0,
                                                    Alu.min, Alu.add)
                            lhs = s_ntv[k]
                        for dq in range(4):
                            nc.tensor.matmul(
                                warped[32 * dq:32 * (dq + 1), :],
                                lhs[:, g, :],
                                tent[:, dq * NK:(dq + 1) * NK],
                                start=True, stop=True,
                                tile_position=(0, 32 * dq))
                        ref_b = s_ref[:, g:g + 1, base:base + 64].broadcast_to(
                            [128, 8, 64])
                        df = wpool.tile([128, NK], f32, tag="df")
                        nc.vector.scalar_tensor_tensor(
                            df[:, :].rearrange("p (a b) -> p a b", a=8),
                            warped[:, :].rearrange("p (a b) -> p a b", a=8),
                            -1.0, ref_b, Alu.mult, Alu.add)
                        adf = wpool.tile([128, NK], f32, tag="adf")
                        nc.vector.scalar_tensor_tensor(
                            adf[:, :], df[:, :], -1.0, df[:, :],
                            Alu.mult, Alu.max)
                        nc.tensor.matmul(outp[:, :],
                                         s_bdp[:, 128 - 4 * g:256 - 4 * g],
                                         adf[:, :], start=(g == 0),
                                         stop=(g == HS - 1))
                    ov = opool.tile([128, NK], f32, tag="ov")
                    nc.vector.tensor_copy(ov[:, :], outp[:, :])
                    nc.sync.dma_start(
                        vol_v[:, :, :, base:base + T],
                        ov[:, :].rearrange("p (dp w) -> p dp w", dp=8)[:, :, :T])
    return nc


def _vertical_rows():
    h = np.arange(H)
    iy = h * (H / (H - 1)) - 0.5
    y0 = np.floor(iy).astype(int)
    wy1 = (iy - y0).astype(np.float32)
    return y0, wy1


def prep_core_inputs(refimg_fea, targetimg_fea, disps, core):
    b = core // 4
    h0 = HS * (core % 4)
    y0, wy1 = _vertical_rows()
    out = {}
    out["dispst"] = np.ascontiguousarray(
        disps[b, :, h0:h0 + HS, :].transpose(1, 0, 2)).astype(np.float32)
    w = np.arange(W, dtype=np.float32)
    out["wrow"] = np.broadcast_to(w * XSCALE - 0.5, (HS, W)).copy()
    tgt = targetimg_fea[b]
    for k in range(NT):
        wp = TOFF[k] + np.arange(128)
        wvalid = (wp >= 0) & (wp < W)
        ga = np.zeros((128, HS, C), np.float32)
        gb = np.zeros((128, HS, C), np.float32)
        tgt_t = np.ascontiguousarray(tgt.transpose(2, 1, 0))  # [W, H, C]
        gh = h0 + np.arange(HS)
        ra, rb = y0[gh], y0[gh] + 1
        rava = (ra >= 0) & (ra < H)
        rbva = (rb >= 0) & (rb < H)
        ga[np.ix_(wvalid, rava)] = tgt_t[wp[wvalid]][:, ra[rava], :]
        gb[np.ix_(wvalid, rbva)] = tgt_t[wp[wvalid]][:, rb[rbva], :]
        out[f"tga{k}"] = ga
        out[f"tgb{k}"] = gb
    out["wyb"] = np.broadcast_to(wy1[h0:h0 + HS], (128, HS)).copy()
    out["refrep"] = np.tile(refimg_fea[b, :, h0:h0 + HS, :],
                            (4, 1, 1)).astype(np.float32)
    sel = np.zeros((HS, HS * 128), np.float32)
    for k in range(HS):
        sel[k, (HS - 1 - k) * 128:(HS - k) * 128] = 1.0
    out["selpad"] = sel
    p = np.arange(128, dtype=np.float32)
    out["wpb"] = np.stack([-(TOFF[k] + p) for k in range(NT)],
                          axis=1).astype(np.float32)
    bdpm = np.zeros((128, 256), np.float32)
    bdpm[np.arange(128), 128 + np.arange(128) // 32] = 1.0
    out["bdp"] = bdpm
    return out


_NC_CACHE = {}


def _get_nc(act8=8, reps=1):
    key = (act8, reps)
    if key not in _NC_CACHE:
        _NC_CACHE[key] = build_nc(act8=act8, reps=reps)
    return _NC_CACHE[key]


def run(refimg_fea, targetimg_fea, disps, act8=8, reps=1):
    from concourse.bass_utils import run_bass_kernel_spmd
    nc = _get_nc(act8=act8, reps=reps)
    in_maps = [prep_core_inputs(refimg_fea, targetimg_fea, disps, core)
               for core in range(NCORES)]
    res = run_bass_kernel_spmd(nc, in_maps, core_ids=list(range(NCORES)))
    full = np.empty((B, D, H, W), np.float32)
    for core in range(NCORES):
        b = core // 4
        h0 = HS * (core % 4)
        full[b, :, h0:h0 + HS, :] = res.results[core]["vol"]
    return full


def kernel(refimg_fea, targetimg_fea, disps):
    refimg_fea = np.asarray(refimg_fea, dtype=np.float32)
    targetimg_fea = np.asarray(targetimg_fea, dtype=np.float32)
    disps = np.asarray(disps, dtype=np.float32)
    return run(refimg_fea, targetimg_fea, disps)
